# revision 13
# baseline (speedup 1.0000x reference)
"""Trainium2 Bass kernel for nn_BlockAttnResTransformerBlock.

Two sequential inter-block-attention sub-layers over 9 blocks (8 completed +
partial).  Per token t (8192 tokens, data-parallel over 8 cores):
  logit_n = <q, V_n[t]> * rsqrt(mean(V_n[t]^2) + eps) / sqrt(D)
  w_n     = exp(logit_n)            (softmax normalizer skipped: rmsnorm of h
                                     is scale-invariant, so it cancels)
  h       = sum_n w_n V_n[t]
  out     = partial[t] + rmsnorm(h) @ W_eff.T    (W_eff = W * norm_w, folded)
Phase 2 repeats with the updated partial and the mlp query/weights.

Engine split (per 128-token tile):
  DVE: 18+1 fused dot-products (scalar_tensor_tensor with accum_out),
       reciprocal for rsqrt, small logit muls
  ACT: 9+1 sum-of-squares (Square + accum), exp, diag(w) builds, h-norm
  PE : diag h-build matmuls, the two DxD GEMMs, residual add folded into the
       GEMM PSUM accumulation via an identity matmul
V ships in ONE bf16 layout (natural [t, n, d], partial packed as block 8);
output is written f32 straight from PSUM.
"""

import numpy as np
import ml_dtypes
from contextlib import ExitStack

import concourse.bass as bass
import concourse.bacc as bacc
import concourse.tile as tile
from concourse import mybir
from concourse.bass_utils import run_bass_kernel_spmd
from concourse.masks import make_identity

bf16 = ml_dtypes.bfloat16

N_BLK = 8          # completed blocks
NB = 9             # + the partial block
B, T, D = 4, 2048, 1024
NCORES = 8
TOK = B * T                  # 8192
TPC = TOK // NCORES          # 1024 tokens per core
NT = TPC // 128              # 8 token-tiles per core
NCH = D // 128               # 8 d-chunks
EPS = 1e-6
INV_SCALE = 1.0 / 32.0       # 1/sqrt(D)

_BF = mybir.dt.bfloat16
_F32 = mybir.dt.float32

_CACHE = {}


def build_nc():
    nc = bacc.Bacc("TRN2", target_bir_lowering=False, debug=False)

    vn = nc.dram_tensor("vn", [NT, 128, NB, D], _BF, kind="ExternalInput")
    qb = nc.dram_tensor("qb", [2, D], _BF, kind="ExternalInput")
    wa = nc.dram_tensor("wa", [128, NCH, D], _BF, kind="ExternalInput")
    wm = nc.dram_tensor("wm", [128, NCH, D], _BF, kind="ExternalInput")
    out = nc.dram_tensor("out", [NT, 128, D], _F32, kind="ExternalOutput")

    AF = mybir.ActivationFunctionType
    OP = mybir.AluOpType

    with tile.TileContext(nc) as tc, ExitStack() as ctx:
        consts = ctx.enter_context(tc.tile_pool(name="consts", bufs=1))
        vin = ctx.enter_context(tc.tile_pool(name="vin", bufs=3))
        stats = ctx.enter_context(tc.tile_pool(name="stats", bufs=4))
        work = ctx.enter_context(tc.tile_pool(name="work", bufs=2))
        pbig = ctx.enter_context(tc.tile_pool(name="pbig", bufs=2, space="PSUM"))

        ident = consts.tile([128, 128], _BF)
        make_identity(nc, ident)
        ident9 = consts.tile([128, NB, 128], _BF)
        for n in range(NB):
            nc.vector.tensor_copy(out=ident9[:, n, :], in_=ident)
        eps_sb = consts.tile([128, 1], _F32)
        nc.vector.memset(eps_sb, EPS)
        qbc = consts.tile([128, 2, D], _BF)
        qb_ap = qb[:, :]
        nc.sync.dma_start(out=qbc, in_=bass.AP(
            tensor=qb_ap.tensor, offset=qb_ap.offset,
            ap=[[0, 128]] + list(qb_ap.ap)))
        wa_sb = consts.tile([128, NCH, D], _BF)
        nc.sync.dma_start(out=wa_sb, in_=wa[:, :, :])
        wm_sb = consts.tile([128, NCH, D], _BF)
        nc.sync.dma_start(out=wm_sb, in_=wm[:, :, :])

        _I32 = mybir.dt.int32

        def rsqrt_dve(src_ap, w, tag, out_scale=1.0):
            """y ~= out_scale * rsqrt(src) on DVE (Quake seed + 1 Newton).

            src must be safely > 0 (sum of many squares here). out_scale is
            folded into the Newton constants for free."""
            i32 = stats.tile([128, w], _I32, tag=tag + "i")
            nc.vector.tensor_scalar(out=i32, in0=src_ap.bitcast(_I32),
                                    scalar1=1, scalar2=-1,
                                    op0=OP.logical_shift_right,
                                    op1=OP.bitwise_xor)
            y0i = stats.tile([128, w], _I32, tag=tag + "y0")
            nc.vector.tensor_scalar(out=y0i, in0=i32, scalar1=1597463008,
                                    scalar2=None, op0=OP.add)
            ycur = y0i.bitcast(_F32)
            t1 = stats.tile([128, w], _F32, tag=tag + "t")
            ynext = stats.tile([128, w], _F32, tag=tag + "yn")
            nc.vector.tensor_mul(out=t1, in0=ycur, in1=ycur)
            nc.vector.tensor_mul(out=t1, in0=t1, in1=src_ap)
            nc.vector.tensor_scalar(out=t1, in0=t1,
                                    scalar1=-0.5 * out_scale,
                                    scalar2=1.5 * out_scale,
                                    op0=OP.mult, op1=OP.add)
            nc.vector.tensor_mul(out=ynext, in0=ycur, in1=t1)
            return ynext

        state = {}

        def emit_load_stats(tt):
            v = vin.tile([128, NB, D], _BF, tag="v")
            for part in range(3):
                ns = slice(3 * part, 3 * part + 3)
                nc.sync.dma_start(out=v[:, ns, :], in_=vn[tt][:, ns, :])
            ssq = stats.tile([128, NB], _F32, tag="ssq")
            dots = stats.tile([128, 2, NB], _F32, tag="dots")

            # per-block reductions: ssq on ACT, dots split DVE/ACT
            for n in range(NB):
                ja = work.tile([128, D], _BF, tag=f"ja{n % 2}")
                nc.scalar.activation(out=ja, in_=v[:, n, :], func=AF.Square,
                                     accum_out=ssq[:, n:n + 1])
            for qi in range(2):
                for n in range(NB):
                    if (qi * NB + n) % 4 < 3:  # 14 on DVE-stt, 4 via ACT
                        jv = work.tile([128, D], _BF, tag=f"jv{n % 2}")
                        nc.vector.scalar_tensor_tensor(
                            out=jv, in0=v[:, n, :], scalar=1.0,
                            in1=qbc[:, qi, :], op0=OP.mult, op1=OP.mult,
                            accum_out=dots[:, qi, n:n + 1])
                    else:
                        pr = work.tile([128, D], _BF, tag=f"pr{n % 2}")
                        nc.vector.tensor_mul(out=pr, in0=v[:, n, :],
                                             in1=qbc[:, qi, :])
                        jb = work.tile([128, D], _BF, tag=f"jb{n % 2}")
                        nc.scalar.activation(out=jb, in_=pr, func=AF.Copy,
                                             accum_out=dots[:, qi, n:n + 1])
            state[tt] = dict(v=v, ssq=ssq, dots=dots, pcur=v[:, 8, :])

        def emit_softmax_diag(tt, phase):
            st = state[tt]
            ssq, dots = st["ssq"], st["dots"]
            # w_n = exp(dot_n * rsqrt(ssq_n/D) / 32) = exp(dot_n*rsqrt(ssq))
            rinv = rsqrt_dve(ssq[:, :], NB, f"r{phase}")
            lg = stats.tile([128, NB], _F32, tag=f"lg{phase}")
            nc.vector.tensor_mul(out=lg, in0=dots[:, phase, :], in1=rinv)
            ex = stats.tile([128, NB], _BF, tag=f"ex{phase}")
            nc.scalar.activation(out=ex, in_=lg, func=AF.Exp)
            # diag(w_n) for all 9 blocks in one broadcast multiply
            diag = work.tile([128, NB, 128], _BF, tag=f"dg{phase}")
            ex_ap = ex[:, :]
            ex_bc = bass.AP(tensor=ex_ap.tensor, offset=ex_ap.offset,
                            ap=list(ex_ap.ap) + [[0, 128]])
            nc.vector.tensor_mul(out=diag, in0=ident9, in1=ex_bc)
            st[f"diag{phase}"] = diag

        def emit_h(tt, phase):
            st = state[tt]
            v, pcur, diag = st["v"], st["pcur"], st[f"diag{phase}"]
            h_ps = pbig.tile([128, D], _F32, tag="h")
            for half in range(2):
                hs = slice(512 * half, 512 * half + 512)
                for n in range(NB):
                    rhs = v[:, n, hs] if n < 8 else pcur[:, hs]
                    nc.tensor.matmul(h_ps[:, hs], lhsT=diag[:, n, :],
                                     rhs=rhs, start=(n == 0), stop=(n == 8))
            st[f"h{phase}"] = h_ps

        def emit_hn(tt, phase):
            st = state[tt]
            h_ps = st[f"h{phase}"]
            # cast h to bf16 unscaled; rmsnorm scale folds into the
            # post-GEMM fused op (GEMM is linear in h)
            ssqh = stats.tile([128, 1], _F32, tag=f"sh{phase}")
            jh = work.tile([128, D], _BF, tag="jh")
            nc.scalar.activation(out=jh, in_=h_ps, func=AF.Square,
                                 accum_out=ssqh)
            rih = rsqrt_dve(ssqh[:, :], 1, f"z{phase}", out_scale=32.0)
            hn = work.tile([128, D], _BF, tag=f"hn{phase}")
            nc.scalar.activation(out=hn, in_=h_ps, func=AF.Copy)
            hnT = work.tile([128, NCH, 128], _BF, tag=f"ht{phase}")
            nc.sync.dma_start_transpose(hnT, hn)
            st[f"rih{phase}"] = rih
            st[f"hnT{phase}"] = hnT

        def emit_gemm(tt, phase):
            st = state[tt]
            hnT = st[f"hnT{phase}"]
            w_sb = wa_sb if phase == 0 else wm_sb
            g_ps = pbig.tile([128, D], _F32, tag="g")
            for half in range(2):
                hs = slice(512 * half, 512 * half + 512)
                for c in range(NCH):
                    nc.tensor.matmul(g_ps[:, hs], lhsT=hnT[:, c, :],
                                     rhs=w_sb[:, c, hs],
                                     start=(c == 0), stop=(c == NCH - 1))
            st[f"g{phase}"] = g_ps

        def emit_pout(tt, phase):
            st = state[tt]
            g_ps, rih = st[f"g{phase}"], st[f"rih{phase}"]
            pcur = st["pcur"]
            ssq, dots = st["ssq"], st["dots"]
            if phase == 0:
                # p1 in bf16 directly; it is the phase-2 residual base
                p1 = work.tile([128, D], _BF, tag="p1")
                nc.vector.scalar_tensor_tensor(
                    out=p1, in0=g_ps, scalar=rih[:, :], in1=pcur,
                    op0=OP.mult, op1=OP.add)
                # refresh block-8 stats for phase 2
                ja = work.tile([128, D], _BF, tag="ja0")
                nc.scalar.activation(out=ja, in_=p1, func=AF.Square,
                                     accum_out=ssq[:, 8:9])
                jv = work.tile([128, D], _BF, tag="jv0")
                nc.vector.scalar_tensor_tensor(
                    out=jv, in0=p1, scalar=1.0, in1=qbc[:, 1, :],
                    op0=OP.mult, op1=OP.mult,
                    accum_out=dots[:, 1, 8:9])
                st["pcur"] = p1
            else:
                pout = work.tile([128, D], _F32, tag="po1")
                nc.vector.scalar_tensor_tensor(
                    out=pout, in0=g_ps, scalar=rih[:, :], in1=pcur,
                    op0=OP.mult, op1=OP.add)
                nc.sync.dma_start(out=out[tt], in_=pout)
                del state[tt]

        # software pipeline over tiles: stats(i) | phase0(i-1) | phase1(i-2)
        # with per-engine interleaving of the two in-flight phases
        for i in range(NT + 2):
            a = i - 1   # tile in phase 0
            b = i - 2   # tile in phase 1
            if 0 <= a < NT:
                emit_softmax_diag(a, 0)
            if 0 <= b < NT:
                emit_softmax_diag(b, 1)
            if 0 <= a < NT:
                emit_h(a, 0)
            if 0 <= b < NT:
                emit_h(b, 1)
            if i < NT:
                emit_load_stats(i)
            if 0 <= a < NT:
                emit_hn(a, 0)
            if 0 <= b < NT:
                emit_hn(b, 1)
            if 0 <= a < NT:
                emit_gemm(a, 0)
            if 0 <= b < NT:
                emit_gemm(b, 1)
            if 0 <= a < NT:
                emit_pout(a, 0)
            if 0 <= b < NT:
                emit_pout(b, 1)

    nc.compile()
    return nc


def _get_nc():
    if "nc" not in _CACHE:
        _CACHE["nc"] = build_nc()
    return _CACHE["nc"]


def _prepare_in_maps(completed_blocks, partial_block, attn_norm_w, attn_w,
                     mlp_norm_w, mlp_w, attn_res_query, attn_res_norm_w,
                     mlp_res_query, mlp_res_norm_w):
    V = np.ascontiguousarray(
        np.asarray(completed_blocks, np.float32)).reshape(N_BLK, TOK, D)
    P = np.ascontiguousarray(
        np.asarray(partial_block, np.float32)).reshape(TOK, D)
    qwa = np.asarray(attn_res_query, np.float32) * np.asarray(attn_res_norm_w, np.float32)
    qwm = np.asarray(mlp_res_query, np.float32) * np.asarray(mlp_res_norm_w, np.float32)
    WaT = (np.asarray(attn_w, np.float32) * np.asarray(attn_norm_w, np.float32)[None, :]).T
    WmT = (np.asarray(mlp_w, np.float32) * np.asarray(mlp_norm_w, np.float32)[None, :]).T

    qb_host = np.ascontiguousarray(
        np.stack([qwa, qwm], axis=0).astype(bf16))            # [2, D]
    wa_host = np.ascontiguousarray(
        WaT.astype(bf16).reshape(NCH, 128, D).transpose(1, 0, 2))
    wm_host = np.ascontiguousarray(
        WmT.astype(bf16).reshape(NCH, 128, D).transpose(1, 0, 2))

    in_maps = []
    for c in range(NCORES):
        sl = slice(c * TPC, (c + 1) * TPC)
        Vc = V[:, sl, :].astype(bf16)                          # [8, 1024, 1024]
        Pc = P[sl].astype(bf16)                                # [1024, 1024]
        vn_host = np.empty((NT, 128, NB, D), dtype=bf16)
        vn_host[:, :, :8, :] = Vc.reshape(N_BLK, NT, 128, D).transpose(1, 2, 0, 3)
        vn_host[:, :, 8, :] = Pc.reshape(NT, 128, D)
        in_maps.append(dict(vn=vn_host, qb=qb_host, wa=wa_host, wm=wm_host))
    return in_maps


def _run(in_maps, **kw):
    nc = _get_nc()
    return run_bass_kernel_spmd(nc, in_maps, core_ids=list(range(NCORES)), **kw)


def kernel(completed_blocks, partial_block, attn_norm_w, attn_w, mlp_norm_w,
           mlp_w, attn_res_query, attn_res_norm_w, mlp_res_query,
           mlp_res_norm_w, layer_in_block=None, **_unused):
    in_maps = _prepare_in_maps(completed_blocks, partial_block, attn_norm_w,
                               attn_w, mlp_norm_w, mlp_w, attn_res_query,
                               attn_res_norm_w, mlp_res_query, mlp_res_norm_w)
    res = _run(in_maps)
    outs = [np.asarray(r["out"], np.float32).reshape(TPC, D) for r in res.results]
    return np.concatenate(outs, axis=0).reshape(B, T, D)


# revision 14
# speedup vs baseline: 1.0244x; 1.0244x over previous
"""Trainium2 Bass kernel for nn_BlockAttnResTransformerBlock.

Two sequential inter-block-attention sub-layers over 9 blocks (8 completed +
partial).  Per token t (8192 tokens, data-parallel over 8 cores):
  logit_n = <q, V_n[t]> * rsqrt(mean(V_n[t]^2) + eps) / sqrt(D)
  w_n     = exp(logit_n)            (softmax normalizer skipped: rmsnorm of h
                                     is scale-invariant, so it cancels)
  h       = sum_n w_n V_n[t]
  out     = partial[t] + rmsnorm(h) @ W_eff.T    (W_eff = W * norm_w, folded)
Phase 2 repeats with the updated partial and the mlp query/weights.

Engine split (per 128-token tile):
  DVE: 18+1 fused dot-products (scalar_tensor_tensor with accum_out),
       reciprocal for rsqrt, small logit muls
  ACT: 9+1 sum-of-squares (Square + accum), exp, diag(w) builds, h-norm
  PE : diag h-build matmuls, the two DxD GEMMs, residual add folded into the
       GEMM PSUM accumulation via an identity matmul
V ships in ONE bf16 layout (natural [t, n, d], partial packed as block 8);
output is written f32 straight from PSUM.
"""

import numpy as np
import ml_dtypes
from contextlib import ExitStack

import concourse.bass as bass
import concourse.bacc as bacc
import concourse.tile as tile
from concourse import mybir
from concourse.bass_utils import run_bass_kernel_spmd
from concourse.masks import make_identity

bf16 = ml_dtypes.bfloat16

N_BLK = 8          # completed blocks
NB = 9             # + the partial block
B, T, D = 4, 2048, 1024
NCORES = 8
TOK = B * T                  # 8192
TPC = TOK // NCORES          # 1024 tokens per core
NT = TPC // 128              # 8 token-tiles per core
NCH = D // 128               # 8 d-chunks
EPS = 1e-6
INV_SCALE = 1.0 / 32.0       # 1/sqrt(D)

_BF = mybir.dt.bfloat16
_F32 = mybir.dt.float32

_CACHE = {}


def build_nc():
    nc = bacc.Bacc("TRN2", target_bir_lowering=False, debug=False)

    vn = nc.dram_tensor("vn", [NT, 128, NB, D], _BF, kind="ExternalInput")
    qb = nc.dram_tensor("qb", [2, D], _BF, kind="ExternalInput")
    wa = nc.dram_tensor("wa", [128, NCH, D], _BF, kind="ExternalInput")
    wm = nc.dram_tensor("wm", [128, NCH, D], _BF, kind="ExternalInput")
    out = nc.dram_tensor("out", [NT, 128, D], _F32, kind="ExternalOutput")

    AF = mybir.ActivationFunctionType
    OP = mybir.AluOpType

    with tile.TileContext(nc) as tc, ExitStack() as ctx:
        consts = ctx.enter_context(tc.tile_pool(name="consts", bufs=1))
        vin = ctx.enter_context(tc.tile_pool(name="vin", bufs=4))
        stats = ctx.enter_context(tc.tile_pool(name="stats", bufs=4))
        work = ctx.enter_context(tc.tile_pool(name="work", bufs=2))
        pbig = ctx.enter_context(tc.tile_pool(name="pbig", bufs=2, space="PSUM"))

        ident = consts.tile([128, 128], _BF)
        make_identity(nc, ident)
        ident9 = consts.tile([128, NB, 128], _BF)
        for n in range(NB):
            nc.vector.tensor_copy(out=ident9[:, n, :], in_=ident)
        eps_sb = consts.tile([128, 1], _F32)
        nc.vector.memset(eps_sb, EPS)
        qbc = consts.tile([128, 2, D], _BF)
        qb_ap = qb[:, :]
        nc.sync.dma_start(out=qbc, in_=bass.AP(
            tensor=qb_ap.tensor, offset=qb_ap.offset,
            ap=[[0, 128]] + list(qb_ap.ap)))
        wa_sb = consts.tile([128, NCH, D], _BF)
        nc.sync.dma_start(out=wa_sb, in_=wa[:, :, :])
        wm_sb = consts.tile([128, NCH, D], _BF)
        nc.sync.dma_start(out=wm_sb, in_=wm[:, :, :])

        _I32 = mybir.dt.int32

        def rsqrt_dve(src_ap, w, tag, out_scale=1.0):
            """y ~= out_scale * rsqrt(src) on DVE (Quake seed + 1 Newton).

            src must be safely > 0 (sum of many squares here). out_scale is
            folded into the Newton constants for free."""
            i32 = stats.tile([128, w], _I32, tag=tag + "i")
            nc.vector.tensor_scalar(out=i32, in0=src_ap.bitcast(_I32),
                                    scalar1=1, scalar2=-1,
                                    op0=OP.logical_shift_right,
                                    op1=OP.bitwise_xor)
            y0i = stats.tile([128, w], _I32, tag=tag + "y0")
            nc.vector.tensor_scalar(out=y0i, in0=i32, scalar1=1597463008,
                                    scalar2=None, op0=OP.add)
            ycur = y0i.bitcast(_F32)
            t1 = stats.tile([128, w], _F32, tag=tag + "t")
            ynext = stats.tile([128, w], _F32, tag=tag + "yn")
            nc.vector.tensor_mul(out=t1, in0=ycur, in1=ycur)
            nc.vector.tensor_mul(out=t1, in0=t1, in1=src_ap)
            nc.vector.tensor_scalar(out=t1, in0=t1,
                                    scalar1=-0.5 * out_scale,
                                    scalar2=1.5 * out_scale,
                                    op0=OP.mult, op1=OP.add)
            nc.vector.tensor_mul(out=ynext, in0=ycur, in1=t1)
            return ynext

        state = {}

        def emit_load_stats(tt):
            v = vin.tile([128, NB, D], _BF, tag="v")
            for part in range(3):
                ns = slice(3 * part, 3 * part + 3)
                nc.sync.dma_start(out=v[:, ns, :], in_=vn[tt][:, ns, :])
            ssq = stats.tile([128, NB], _F32, tag="ssq")
            dots = stats.tile([128, 2, NB], _F32, tag="dots")

            # per-block reductions: ssq on ACT, dots split DVE/ACT
            for n in range(NB):
                ja = work.tile([128, D], _BF, tag=f"ja{n % 2}")
                nc.scalar.activation(out=ja, in_=v[:, n, :], func=AF.Square,
                                     accum_out=ssq[:, n:n + 1])
            for qi in range(2):
                for n in range(NB):
                    if (qi * NB + n) % 4 < 3:  # 14 on DVE-stt, 4 via ACT
                        jv = work.tile([128, D], _BF, tag=f"jv{n % 2}")
                        nc.vector.scalar_tensor_tensor(
                            out=jv, in0=v[:, n, :], scalar=1.0,
                            in1=qbc[:, qi, :], op0=OP.mult, op1=OP.mult,
                            accum_out=dots[:, qi, n:n + 1])
                    else:
                        pr = work.tile([128, D], _BF, tag=f"pv{n % 2}", bufs=2)
                        nc.vector.tensor_mul(out=pr, in0=v[:, n, :],
                                             in1=qbc[:, qi, :])
                        jb = work.tile([128, D], _BF, tag=f"ja{n % 2}")
                        nc.scalar.activation(out=jb, in_=pr, func=AF.Copy,
                                             accum_out=dots[:, qi, n:n + 1])
            state[tt] = dict(v=v, ssq=ssq, dots=dots, pcur=v[:, 8, :])

        def emit_softmax_diag(tt, phase):
            st = state[tt]
            ssq, dots = st["ssq"], st["dots"]
            # w_n = exp(dot_n * rsqrt(ssq_n/D) / 32) = exp(dot_n*rsqrt(ssq))
            rinv = rsqrt_dve(ssq[:, :], NB, f"r{phase}")
            lg = stats.tile([128, NB], _F32, tag=f"lg{phase}")
            nc.vector.tensor_mul(out=lg, in0=dots[:, phase, :], in1=rinv)
            ex = stats.tile([128, NB], _BF, tag=f"ex{phase}")
            nc.scalar.activation(out=ex, in_=lg, func=AF.Exp)
            # diag(w_n) for all 9 blocks in one broadcast multiply
            diag = work.tile([128, NB, 128], _BF, tag=f"dg{phase}")
            ex_ap = ex[:, :]
            ex_bc = bass.AP(tensor=ex_ap.tensor, offset=ex_ap.offset,
                            ap=list(ex_ap.ap) + [[0, 128]])
            nc.vector.tensor_mul(out=diag, in0=ident9, in1=ex_bc)
            st[f"diag{phase}"] = diag

        def emit_h(tt, phase):
            st = state[tt]
            v, pcur, diag = st["v"], st["pcur"], st[f"diag{phase}"]
            h_ps = pbig.tile([128, D], _F32, tag="h")
            for half in range(2):
                hs = slice(512 * half, 512 * half + 512)
                for n in range(NB):
                    rhs = v[:, n, hs] if n < 8 else pcur[:, hs]
                    nc.tensor.matmul(h_ps[:, hs], lhsT=diag[:, n, :],
                                     rhs=rhs, start=(n == 0), stop=(n == 8))
            st[f"h{phase}"] = h_ps

        def emit_hn(tt, phase):
            st = state[tt]
            h_ps = st[f"h{phase}"]
            # cast h to bf16 unscaled; rmsnorm scale folds into the
            # post-GEMM fused op (GEMM is linear in h)
            ssqh = stats.tile([128, 1], _F32, tag=f"sh{phase}")
            jh = work.tile([128, D], _BF, tag="jh")
            nc.scalar.activation(out=jh, in_=h_ps, func=AF.Square,
                                 accum_out=ssqh)
            rih = rsqrt_dve(ssqh[:, :], 1, f"z{phase}", out_scale=32.0)
            hn = work.tile([128, D], _BF, tag=f"hn{phase}")
            nc.scalar.activation(out=hn, in_=h_ps, func=AF.Copy)
            hnT = work.tile([128, NCH, 128], _BF, tag=f"ht{phase}")
            nc.sync.dma_start_transpose(hnT, hn)
            st[f"rih{phase}"] = rih
            st[f"hnT{phase}"] = hnT

        def emit_gemm(tt, phase):
            st = state[tt]
            hnT = st[f"hnT{phase}"]
            w_sb = wa_sb if phase == 0 else wm_sb
            g_ps = pbig.tile([128, D], _F32, tag="g")
            for half in range(2):
                hs = slice(512 * half, 512 * half + 512)
                for c in range(NCH):
                    nc.tensor.matmul(g_ps[:, hs], lhsT=hnT[:, c, :],
                                     rhs=w_sb[:, c, hs],
                                     start=(c == 0), stop=(c == NCH - 1))
            st[f"g{phase}"] = g_ps

        def emit_pout(tt, phase):
            st = state[tt]
            g_ps, rih = st[f"g{phase}"], st[f"rih{phase}"]
            pcur = st["pcur"]
            ssq, dots = st["ssq"], st["dots"]
            if phase == 0:
                # p1 in bf16 directly; it is the phase-2 residual base
                p1 = work.tile([128, D], _BF, tag="p1")
                nc.vector.scalar_tensor_tensor(
                    out=p1, in0=g_ps, scalar=rih[:, :], in1=pcur,
                    op0=OP.mult, op1=OP.add)
                # refresh block-8 stats for phase 2
                ja = work.tile([128, D], _BF, tag="ja0")
                nc.scalar.activation(out=ja, in_=p1, func=AF.Square,
                                     accum_out=ssq[:, 8:9])
                jv = work.tile([128, D], _BF, tag="jv0")
                nc.vector.scalar_tensor_tensor(
                    out=jv, in0=p1, scalar=1.0, in1=qbc[:, 1, :],
                    op0=OP.mult, op1=OP.mult,
                    accum_out=dots[:, 1, 8:9])
                st["pcur"] = p1
            else:
                pout = work.tile([128, D], _F32, tag="po1")
                nc.vector.scalar_tensor_tensor(
                    out=pout, in0=g_ps, scalar=rih[:, :], in1=pcur,
                    op0=OP.mult, op1=OP.add)
                nc.sync.dma_start(out=out[tt], in_=pout)
                del state[tt]

        # software pipeline over tiles: stats(i) | phase0(i-1) | phase1(i-2)
        # with per-engine interleaving of the two in-flight phases
        for i in range(NT + 2):
            a = i - 1   # tile in phase 0
            b = i - 2   # tile in phase 1
            if 0 <= a < NT:
                emit_softmax_diag(a, 0)
            if 0 <= b < NT:
                emit_softmax_diag(b, 1)
            if 0 <= a < NT:
                emit_h(a, 0)
            if 0 <= b < NT:
                emit_h(b, 1)
            if i < NT:
                emit_load_stats(i)
            if 0 <= a < NT:
                emit_hn(a, 0)
            if 0 <= b < NT:
                emit_hn(b, 1)
            if 0 <= a < NT:
                emit_gemm(a, 0)
            if 0 <= b < NT:
                emit_gemm(b, 1)
            if 0 <= a < NT:
                emit_pout(a, 0)
            if 0 <= b < NT:
                emit_pout(b, 1)

    nc.compile()
    return nc


def _get_nc():
    if "nc" not in _CACHE:
        _CACHE["nc"] = build_nc()
    return _CACHE["nc"]


def _prepare_in_maps(completed_blocks, partial_block, attn_norm_w, attn_w,
                     mlp_norm_w, mlp_w, attn_res_query, attn_res_norm_w,
                     mlp_res_query, mlp_res_norm_w):
    V = np.ascontiguousarray(
        np.asarray(completed_blocks, np.float32)).reshape(N_BLK, TOK, D)
    P = np.ascontiguousarray(
        np.asarray(partial_block, np.float32)).reshape(TOK, D)
    qwa = np.asarray(attn_res_query, np.float32) * np.asarray(attn_res_norm_w, np.float32)
    qwm = np.asarray(mlp_res_query, np.float32) * np.asarray(mlp_res_norm_w, np.float32)
    WaT = (np.asarray(attn_w, np.float32) * np.asarray(attn_norm_w, np.float32)[None, :]).T
    WmT = (np.asarray(mlp_w, np.float32) * np.asarray(mlp_norm_w, np.float32)[None, :]).T

    qb_host = np.ascontiguousarray(
        np.stack([qwa, qwm], axis=0).astype(bf16))            # [2, D]
    wa_host = np.ascontiguousarray(
        WaT.astype(bf16).reshape(NCH, 128, D).transpose(1, 0, 2))
    wm_host = np.ascontiguousarray(
        WmT.astype(bf16).reshape(NCH, 128, D).transpose(1, 0, 2))

    in_maps = []
    for c in range(NCORES):
        sl = slice(c * TPC, (c + 1) * TPC)
        Vc = V[:, sl, :].astype(bf16)                          # [8, 1024, 1024]
        Pc = P[sl].astype(bf16)                                # [1024, 1024]
        vn_host = np.empty((NT, 128, NB, D), dtype=bf16)
        vn_host[:, :, :8, :] = Vc.reshape(N_BLK, NT, 128, D).transpose(1, 2, 0, 3)
        vn_host[:, :, 8, :] = Pc.reshape(NT, 128, D)
        in_maps.append(dict(vn=vn_host, qb=qb_host, wa=wa_host, wm=wm_host))
    return in_maps


def _run(in_maps, **kw):
    nc = _get_nc()
    return run_bass_kernel_spmd(nc, in_maps, core_ids=list(range(NCORES)), **kw)


def kernel(completed_blocks, partial_block, attn_norm_w, attn_w, mlp_norm_w,
           mlp_w, attn_res_query, attn_res_norm_w, mlp_res_query,
           mlp_res_norm_w, layer_in_block=None, **_unused):
    in_maps = _prepare_in_maps(completed_blocks, partial_block, attn_norm_w,
                               attn_w, mlp_norm_w, mlp_w, attn_res_query,
                               attn_res_norm_w, mlp_res_query, mlp_res_norm_w)
    res = _run(in_maps)
    outs = [np.asarray(r["out"], np.float32).reshape(TPC, D) for r in res.results]
    return np.concatenate(outs, axis=0).reshape(B, T, D)


# revision 15
# speedup vs baseline: 1.0273x; 1.0029x over previous
"""Trainium2 Bass kernel for nn_BlockAttnResTransformerBlock.

Two sequential inter-block-attention sub-layers over 9 blocks (8 completed +
partial).  Per token t (8192 tokens, data-parallel over 8 cores):
  logit_n = <q, V_n[t]> * rsqrt(mean(V_n[t]^2) + eps) / sqrt(D)
  w_n     = exp(logit_n)            (softmax normalizer skipped: rmsnorm of h
                                     is scale-invariant, so it cancels)
  h       = sum_n w_n V_n[t]
  out     = partial[t] + rmsnorm(h) @ W_eff.T    (W_eff = W * norm_w, folded)
Phase 2 repeats with the updated partial and the mlp query/weights.

Engine split (per 128-token tile):
  DVE: 18+1 fused dot-products (scalar_tensor_tensor with accum_out),
       reciprocal for rsqrt, small logit muls
  ACT: 9+1 sum-of-squares (Square + accum), exp, diag(w) builds, h-norm
  PE : diag h-build matmuls, the two DxD GEMMs, residual add folded into the
       GEMM PSUM accumulation via an identity matmul
V ships in ONE bf16 layout (natural [t, n, d], partial packed as block 8);
output is written f32 straight from PSUM.
"""

import numpy as np
import ml_dtypes
from contextlib import ExitStack

import concourse.bass as bass
import concourse.bacc as bacc
import concourse.tile as tile
from concourse import mybir
from concourse.bass_utils import run_bass_kernel_spmd
from concourse.masks import make_identity

bf16 = ml_dtypes.bfloat16

N_BLK = 8          # completed blocks
NB = 9             # + the partial block
B, T, D = 4, 2048, 1024
NCORES = 8
TOK = B * T                  # 8192
TPC = TOK // NCORES          # 1024 tokens per core
NT = TPC // 128              # 8 token-tiles per core
NCH = D // 128               # 8 d-chunks
EPS = 1e-6
INV_SCALE = 1.0 / 32.0       # 1/sqrt(D)

_BF = mybir.dt.bfloat16
_F32 = mybir.dt.float32

_CACHE = {}


def build_nc():
    nc = bacc.Bacc("TRN2", target_bir_lowering=False, debug=False)

    vn = nc.dram_tensor("vn", [NT, 128, NB, D], _BF, kind="ExternalInput")
    qb = nc.dram_tensor("qb", [2, D], _BF, kind="ExternalInput")
    wa = nc.dram_tensor("wa", [128, NCH, D], _BF, kind="ExternalInput")
    wm = nc.dram_tensor("wm", [128, NCH, D], _BF, kind="ExternalInput")
    out = nc.dram_tensor("out", [NT, 128, D], _F32, kind="ExternalOutput")

    AF = mybir.ActivationFunctionType
    OP = mybir.AluOpType

    with tile.TileContext(nc) as tc, ExitStack() as ctx:
        consts = ctx.enter_context(tc.tile_pool(name="consts", bufs=1))
        vin = ctx.enter_context(tc.tile_pool(name="vin", bufs=4))
        stats = ctx.enter_context(tc.tile_pool(name="stats", bufs=4))
        work = ctx.enter_context(tc.tile_pool(name="work", bufs=2))
        pbig = ctx.enter_context(tc.tile_pool(name="pbig", bufs=2, space="PSUM"))

        ident = consts.tile([128, 128], _BF)
        make_identity(nc, ident)
        ident9 = consts.tile([128, NB, 128], _BF)
        for n in range(NB):
            nc.scalar.activation(out=ident9[:, n, :], in_=ident, func=AF.Copy)
        eps_sb = consts.tile([128, 1], _F32)
        nc.vector.memset(eps_sb, EPS)
        qbc = consts.tile([128, 2, D], _BF)
        qb_ap = qb[:, :]
        nc.sync.dma_start(out=qbc, in_=bass.AP(
            tensor=qb_ap.tensor, offset=qb_ap.offset,
            ap=[[0, 128]] + list(qb_ap.ap)))
        wa_sb = consts.tile([128, NCH, D], _BF)
        nc.sync.dma_start(out=wa_sb, in_=wa[:, :, :])
        wm_sb = consts.tile([128, NCH, D], _BF)
        nc.sync.dma_start(out=wm_sb, in_=wm[:, :, :])

        _I32 = mybir.dt.int32

        def rsqrt_dve(src_ap, w, tag, out_scale=1.0):
            """y ~= out_scale * rsqrt(src) on DVE (Quake seed + 1 Newton).

            src must be safely > 0 (sum of many squares here). out_scale is
            folded into the Newton constants for free."""
            i32 = stats.tile([128, w], _I32, tag=tag + "i")
            nc.vector.tensor_scalar(out=i32, in0=src_ap.bitcast(_I32),
                                    scalar1=1, scalar2=-1,
                                    op0=OP.logical_shift_right,
                                    op1=OP.bitwise_xor)
            y0i = stats.tile([128, w], _I32, tag=tag + "y0")
            nc.vector.tensor_scalar(out=y0i, in0=i32, scalar1=1597463008,
                                    scalar2=None, op0=OP.add)
            ycur = y0i.bitcast(_F32)
            t1 = stats.tile([128, w], _F32, tag=tag + "t")
            ynext = stats.tile([128, w], _F32, tag=tag + "yn")
            nc.vector.tensor_mul(out=t1, in0=ycur, in1=ycur)
            nc.vector.tensor_mul(out=t1, in0=t1, in1=src_ap)
            nc.vector.tensor_scalar(out=t1, in0=t1,
                                    scalar1=-0.5 * out_scale,
                                    scalar2=1.5 * out_scale,
                                    op0=OP.mult, op1=OP.add)
            nc.vector.tensor_mul(out=ynext, in0=ycur, in1=t1)
            return ynext

        state = {}

        def emit_load_stats(tt):
            v = vin.tile([128, NB, D], _BF, tag="v")
            for part in range(3):
                ns = slice(3 * part, 3 * part + 3)
                nc.sync.dma_start(out=v[:, ns, :], in_=vn[tt][:, ns, :])
            ssq = stats.tile([128, NB], _F32, tag="ssq")
            dots = stats.tile([128, 2, NB], _F32, tag="dots")

            # per-block reductions: ssq on ACT, dots split DVE/ACT
            for n in range(NB):
                ja = work.tile([128, D], _BF, tag=f"ja{n % 2}")
                nc.scalar.activation(out=ja, in_=v[:, n, :], func=AF.Square,
                                     accum_out=ssq[:, n:n + 1])
            for qi in range(2):
                for n in range(NB):
                    if (qi * NB + n) % 4 < 3:  # 14 on DVE-stt, 4 via ACT
                        jv = work.tile([128, D], _BF, tag=f"jv{n % 2}")
                        nc.vector.scalar_tensor_tensor(
                            out=jv, in0=v[:, n, :], scalar=1.0,
                            in1=qbc[:, qi, :], op0=OP.mult, op1=OP.mult,
                            accum_out=dots[:, qi, n:n + 1])
                    else:
                        pr = work.tile([128, D], _BF, tag=f"pv{n % 2}", bufs=2)
                        nc.vector.tensor_mul(out=pr, in0=v[:, n, :],
                                             in1=qbc[:, qi, :])
                        jb = work.tile([128, D], _BF, tag=f"ja{n % 2}")
                        nc.scalar.activation(out=jb, in_=pr, func=AF.Copy,
                                             accum_out=dots[:, qi, n:n + 1])
            state[tt] = dict(v=v, ssq=ssq, dots=dots, pcur=v[:, 8, :])

        def emit_softmax_diag(tt, phase):
            st = state[tt]
            ssq, dots = st["ssq"], st["dots"]
            # w_n = exp(dot_n * rsqrt(ssq_n/D) / 32) = exp(dot_n*rsqrt(ssq))
            rinv = rsqrt_dve(ssq[:, :], NB, f"r{phase}")
            lg = stats.tile([128, NB], _F32, tag=f"lg{phase}")
            nc.vector.tensor_mul(out=lg, in0=dots[:, phase, :], in1=rinv)
            ex = stats.tile([128, NB], _F32, tag=f"ex{phase}")
            nc.scalar.activation(out=ex, in_=lg, func=AF.Exp)
            # diag(w_n): ACT copy-scales (phase 0) / DVE broadcast (phase 1)
            diag = work.tile([128, NB, 128], _BF, tag=f"dg{phase}")
            if phase == 0:
                for n in range(NB):
                    nc.scalar.activation(out=diag[:, n, :], in_=ident,
                                         func=AF.Copy, scale=ex[:, n:n + 1])
            else:
                ex_ap = ex[:, :]
                ex_bc = bass.AP(tensor=ex_ap.tensor, offset=ex_ap.offset,
                                ap=list(ex_ap.ap) + [[0, 128]])
                nc.vector.tensor_mul(out=diag, in0=ident9, in1=ex_bc)
            st[f"diag{phase}"] = diag

        def emit_h(tt, phase):
            st = state[tt]
            v, pcur, diag = st["v"], st["pcur"], st[f"diag{phase}"]
            h_ps = pbig.tile([128, D], _F32, tag="h")
            for half in range(2):
                hs = slice(512 * half, 512 * half + 512)
                for n in range(NB):
                    rhs = v[:, n, hs] if n < 8 else pcur[:, hs]
                    nc.tensor.matmul(h_ps[:, hs], lhsT=diag[:, n, :],
                                     rhs=rhs, start=(n == 0), stop=(n == 8))
            st[f"h{phase}"] = h_ps

        def emit_hn(tt, phase):
            st = state[tt]
            h_ps = st[f"h{phase}"]
            # cast h to bf16 unscaled; rmsnorm scale folds into the
            # post-GEMM fused op (GEMM is linear in h)
            ssqh = stats.tile([128, 1], _F32, tag=f"sh{phase}")
            jh = work.tile([128, D], _BF, tag="jh")
            nc.scalar.activation(out=jh, in_=h_ps, func=AF.Square,
                                 accum_out=ssqh)
            rih = rsqrt_dve(ssqh[:, :], 1, f"z{phase}", out_scale=32.0)
            hn = work.tile([128, D], _BF, tag=f"hn{phase}")
            nc.scalar.activation(out=hn, in_=h_ps, func=AF.Copy)
            hnT = work.tile([128, NCH, 128], _BF, tag=f"ht{phase}")
            nc.sync.dma_start_transpose(hnT, hn)
            st[f"rih{phase}"] = rih
            st[f"hnT{phase}"] = hnT

        def emit_gemm(tt, phase):
            st = state[tt]
            hnT = st[f"hnT{phase}"]
            w_sb = wa_sb if phase == 0 else wm_sb
            g_ps = pbig.tile([128, D], _F32, tag="g")
            for half in range(2):
                hs = slice(512 * half, 512 * half + 512)
                for c in range(NCH):
                    nc.tensor.matmul(g_ps[:, hs], lhsT=hnT[:, c, :],
                                     rhs=w_sb[:, c, hs],
                                     start=(c == 0), stop=(c == NCH - 1))
            st[f"g{phase}"] = g_ps

        def emit_pout(tt, phase):
            st = state[tt]
            g_ps, rih = st[f"g{phase}"], st[f"rih{phase}"]
            pcur = st["pcur"]
            ssq, dots = st["ssq"], st["dots"]
            if phase == 0:
                # p1 in bf16 directly; it is the phase-2 residual base
                p1 = work.tile([128, D], _BF, tag="p1")
                nc.vector.scalar_tensor_tensor(
                    out=p1, in0=g_ps, scalar=rih[:, :], in1=pcur,
                    op0=OP.mult, op1=OP.add)
                # refresh block-8 stats for phase 2
                ja = work.tile([128, D], _BF, tag="ja0")
                nc.scalar.activation(out=ja, in_=p1, func=AF.Square,
                                     accum_out=ssq[:, 8:9])
                jv = work.tile([128, D], _BF, tag="jv0")
                nc.vector.scalar_tensor_tensor(
                    out=jv, in0=p1, scalar=1.0, in1=qbc[:, 1, :],
                    op0=OP.mult, op1=OP.mult,
                    accum_out=dots[:, 1, 8:9])
                st["pcur"] = p1
            else:
                pout = work.tile([128, D], _F32, tag="po1")
                nc.vector.scalar_tensor_tensor(
                    out=pout, in0=g_ps, scalar=rih[:, :], in1=pcur,
                    op0=OP.mult, op1=OP.add)
                nc.sync.dma_start(out=out[tt], in_=pout)
                del state[tt]

        # software pipeline over tiles: stats(i) | phase0(i-1) | phase1(i-2)
        # with per-engine interleaving of the two in-flight phases
        for i in range(NT + 2):
            a = i - 1   # tile in phase 0
            b = i - 2   # tile in phase 1
            if 0 <= a < NT:
                emit_softmax_diag(a, 0)
            if 0 <= b < NT:
                emit_softmax_diag(b, 1)
            if 0 <= a < NT:
                emit_h(a, 0)
            if 0 <= b < NT:
                emit_h(b, 1)
            if i < NT:
                emit_load_stats(i)
            if 0 <= a < NT:
                emit_hn(a, 0)
            if 0 <= b < NT:
                emit_hn(b, 1)
            if 0 <= a < NT:
                emit_gemm(a, 0)
            if 0 <= b < NT:
                emit_gemm(b, 1)
            if 0 <= a < NT:
                emit_pout(a, 0)
            if 0 <= b < NT:
                emit_pout(b, 1)

    nc.compile()
    return nc


def _get_nc():
    if "nc" not in _CACHE:
        _CACHE["nc"] = build_nc()
    return _CACHE["nc"]


def _prepare_in_maps(completed_blocks, partial_block, attn_norm_w, attn_w,
                     mlp_norm_w, mlp_w, attn_res_query, attn_res_norm_w,
                     mlp_res_query, mlp_res_norm_w):
    V = np.ascontiguousarray(
        np.asarray(completed_blocks, np.float32)).reshape(N_BLK, TOK, D)
    P = np.ascontiguousarray(
        np.asarray(partial_block, np.float32)).reshape(TOK, D)
    qwa = np.asarray(attn_res_query, np.float32) * np.asarray(attn_res_norm_w, np.float32)
    qwm = np.asarray(mlp_res_query, np.float32) * np.asarray(mlp_res_norm_w, np.float32)
    WaT = (np.asarray(attn_w, np.float32) * np.asarray(attn_norm_w, np.float32)[None, :]).T
    WmT = (np.asarray(mlp_w, np.float32) * np.asarray(mlp_norm_w, np.float32)[None, :]).T

    qb_host = np.ascontiguousarray(
        np.stack([qwa, qwm], axis=0).astype(bf16))            # [2, D]
    wa_host = np.ascontiguousarray(
        WaT.astype(bf16).reshape(NCH, 128, D).transpose(1, 0, 2))
    wm_host = np.ascontiguousarray(
        WmT.astype(bf16).reshape(NCH, 128, D).transpose(1, 0, 2))

    in_maps = []
    for c in range(NCORES):
        sl = slice(c * TPC, (c + 1) * TPC)
        Vc = V[:, sl, :].astype(bf16)                          # [8, 1024, 1024]
        Pc = P[sl].astype(bf16)                                # [1024, 1024]
        vn_host = np.empty((NT, 128, NB, D), dtype=bf16)
        vn_host[:, :, :8, :] = Vc.reshape(N_BLK, NT, 128, D).transpose(1, 2, 0, 3)
        vn_host[:, :, 8, :] = Pc.reshape(NT, 128, D)
        in_maps.append(dict(vn=vn_host, qb=qb_host, wa=wa_host, wm=wm_host))
    return in_maps


def _run(in_maps, **kw):
    nc = _get_nc()
    return run_bass_kernel_spmd(nc, in_maps, core_ids=list(range(NCORES)), **kw)


def kernel(completed_blocks, partial_block, attn_norm_w, attn_w, mlp_norm_w,
           mlp_w, attn_res_query, attn_res_norm_w, mlp_res_query,
           mlp_res_norm_w, layer_in_block=None, **_unused):
    in_maps = _prepare_in_maps(completed_blocks, partial_block, attn_norm_w,
                               attn_w, mlp_norm_w, mlp_w, attn_res_query,
                               attn_res_norm_w, mlp_res_query, mlp_res_norm_w)
    res = _run(in_maps)
    outs = [np.asarray(r["out"], np.float32).reshape(TPC, D) for r in res.results]
    return np.concatenate(outs, axis=0).reshape(B, T, D)


# revision 16
# speedup vs baseline: 1.0543x; 1.0263x over previous
"""Trainium2 Bass kernel for nn_BlockAttnResTransformerBlock.

Two sequential inter-block-attention sub-layers over 9 blocks (8 completed +
partial).  Per token t (8192 tokens, data-parallel over 8 cores):
  logit_n = <q, V_n[t]> * rsqrt(mean(V_n[t]^2) + eps) / sqrt(D)
  w_n     = exp(logit_n)            (softmax normalizer skipped: rmsnorm of h
                                     is scale-invariant, so it cancels)
  h       = sum_n w_n V_n[t]
  out     = partial[t] + rmsnorm(h) @ W_eff.T    (W_eff = W * norm_w, folded)
Phase 2 repeats with the updated partial and the mlp query/weights.

Engine split (per 128-token tile):
  DVE: 18+1 fused dot-products (scalar_tensor_tensor with accum_out),
       reciprocal for rsqrt, small logit muls
  ACT: 9+1 sum-of-squares (Square + accum), exp, diag(w) builds, h-norm
  PE : diag h-build matmuls, the two DxD GEMMs, residual add folded into the
       GEMM PSUM accumulation via an identity matmul
V ships in ONE bf16 layout (natural [t, n, d], partial packed as block 8);
output is written f32 straight from PSUM.
"""

import numpy as np
import ml_dtypes
from contextlib import ExitStack

import concourse.bass as bass
import concourse.bacc as bacc
import concourse.tile as tile
from concourse import mybir
from concourse.bass_utils import run_bass_kernel_spmd
from concourse.masks import make_identity

bf16 = ml_dtypes.bfloat16

N_BLK = 8          # completed blocks
NB = 9             # + the partial block
B, T, D = 4, 2048, 1024
NCORES = 8
TOK = B * T                  # 8192
TPC = TOK // NCORES          # 1024 tokens per core
NT = TPC // 128              # 8 token-tiles per core
NCH = D // 128               # 8 d-chunks
EPS = 1e-6
INV_SCALE = 1.0 / 32.0       # 1/sqrt(D)

_BF = mybir.dt.bfloat16
_F32 = mybir.dt.float32

_CACHE = {}


def build_nc():
    nc = bacc.Bacc("TRN2", target_bir_lowering=False, debug=False)

    vn = nc.dram_tensor("vn", [NT, 128, NB, D], _BF, kind="ExternalInput")
    qb = nc.dram_tensor("qb", [2, D], _BF, kind="ExternalInput")
    wa = nc.dram_tensor("wa", [128, NCH, D], _BF, kind="ExternalInput")
    wm = nc.dram_tensor("wm", [128, NCH, D], _BF, kind="ExternalInput")
    out = nc.dram_tensor("out", [NT, 128, D], _F32, kind="ExternalOutput")

    AF = mybir.ActivationFunctionType
    OP = mybir.AluOpType

    with tile.TileContext(nc) as tc, ExitStack() as ctx:
        consts = ctx.enter_context(tc.tile_pool(name="consts", bufs=1))
        vin = ctx.enter_context(tc.tile_pool(name="vin", bufs=4))
        stats = ctx.enter_context(tc.tile_pool(name="stats", bufs=4))
        work = ctx.enter_context(tc.tile_pool(name="work", bufs=2))
        pbig = ctx.enter_context(tc.tile_pool(name="pbig", bufs=2, space="PSUM"))

        ident = consts.tile([128, 128], _BF)
        make_identity(nc, ident)
        ident9 = consts.tile([128, NB, 128], _BF)
        for n in range(NB):
            nc.scalar.activation(out=ident9[:, n, :], in_=ident, func=AF.Copy)
        qbc = consts.tile([128, 2, D], _BF)
        qb_ap = qb[:, :]
        nc.sync.dma_start(out=qbc, in_=bass.AP(
            tensor=qb_ap.tensor, offset=qb_ap.offset,
            ap=[[0, 128]] + list(qb_ap.ap)))
        wa_sb = consts.tile([128, NCH, D], _BF)
        wm_sb = consts.tile([128, NCH, D], _BF)

        def emit_weight_loads():
            nc.sync.dma_start(out=wa_sb, in_=wa[:, :, :])
            nc.sync.dma_start(out=wm_sb, in_=wm[:, :, :])

        _I32 = mybir.dt.int32

        def rsqrt_dve(src_ap, w, tag, out_scale=1.0):
            """y ~= out_scale * rsqrt(src) on DVE (Quake seed + 1 Newton).

            src must be safely > 0 (sum of many squares here). out_scale is
            folded into the Newton constants for free."""
            i32 = stats.tile([128, w], _I32, tag=tag + "i")
            nc.vector.tensor_scalar(out=i32, in0=src_ap.bitcast(_I32),
                                    scalar1=1, scalar2=-1,
                                    op0=OP.logical_shift_right,
                                    op1=OP.bitwise_xor)
            y0i = stats.tile([128, w], _I32, tag=tag + "y0")
            nc.vector.tensor_scalar(out=y0i, in0=i32, scalar1=1597463008,
                                    scalar2=None, op0=OP.add)
            ycur = y0i.bitcast(_F32)
            t1 = stats.tile([128, w], _F32, tag=tag + "t")
            ynext = stats.tile([128, w], _F32, tag=tag + "yn")
            nc.vector.tensor_mul(out=t1, in0=ycur, in1=ycur)
            nc.vector.tensor_mul(out=t1, in0=t1, in1=src_ap)
            nc.vector.tensor_scalar(out=t1, in0=t1,
                                    scalar1=-0.5 * out_scale,
                                    scalar2=1.5 * out_scale,
                                    op0=OP.mult, op1=OP.add)
            nc.vector.tensor_mul(out=ynext, in0=ycur, in1=t1)
            return ynext

        state = {}

        def emit_load_stats(tt):
            v = vin.tile([128, NB, D], _BF, tag="v")
            for part in range(3):
                ns = slice(3 * part, 3 * part + 3)
                nc.sync.dma_start(out=v[:, ns, :], in_=vn[tt][:, ns, :])
            ssq = stats.tile([128, NB], _F32, tag="ssq")
            dots = stats.tile([128, 2, NB], _F32, tag="dots")

            # per-block reductions: ssq on ACT, dots split DVE/ACT
            for n in range(NB):
                ja = work.tile([128, D], _BF, tag=f"ja{n % 2}")
                nc.scalar.activation(out=ja, in_=v[:, n, :], func=AF.Square,
                                     accum_out=ssq[:, n:n + 1])
            for qi in range(2):
                for n in range(NB):
                    if (qi * NB + n) % 4 < 3:  # 14 on DVE-stt, 4 via ACT
                        jv = work.tile([128, D], _BF, tag=f"jv{n % 2}")
                        nc.vector.scalar_tensor_tensor(
                            out=jv, in0=v[:, n, :], scalar=1.0,
                            in1=qbc[:, qi, :], op0=OP.mult, op1=OP.mult,
                            accum_out=dots[:, qi, n:n + 1])
                    else:
                        pr = work.tile([128, D], _BF, tag=f"pv{n % 2}", bufs=2)
                        nc.vector.tensor_mul(out=pr, in0=v[:, n, :],
                                             in1=qbc[:, qi, :])
                        jb = work.tile([128, D], _BF, tag=f"ja{n % 2}")
                        nc.scalar.activation(out=jb, in_=pr, func=AF.Copy,
                                             accum_out=dots[:, qi, n:n + 1])
            state[tt] = dict(v=v, ssq=ssq, dots=dots, pcur=v[:, 8, :])

        def emit_softmax_diag(tt, phase):
            st = state[tt]
            ssq, dots = st["ssq"], st["dots"]
            # w_n = exp(dot_n * rsqrt(ssq_n/D) / 32) = exp(dot_n*rsqrt(ssq))
            rinv = rsqrt_dve(ssq[:, :], NB, f"r{phase}")
            lg = stats.tile([128, NB], _F32, tag=f"lg{phase}")
            nc.vector.tensor_mul(out=lg, in0=dots[:, phase, :], in1=rinv)
            ex = stats.tile([128, NB], _F32, tag=f"ex{phase}")
            nc.scalar.activation(out=ex, in_=lg, func=AF.Exp)
            # diag(w_n): ACT copy-scales (phase 0) / DVE broadcast (phase 1)
            diag = work.tile([128, NB, 128], _BF, tag=f"dg{phase}", bufs=3)
            if phase == 0:
                for n in range(NB):
                    nc.scalar.activation(out=diag[:, n, :], in_=ident,
                                         func=AF.Copy, scale=ex[:, n:n + 1])
            else:
                ex_ap = ex[:, :]
                ex_bc = bass.AP(tensor=ex_ap.tensor, offset=ex_ap.offset,
                                ap=list(ex_ap.ap) + [[0, 128]])
                nc.vector.tensor_mul(out=diag, in0=ident9, in1=ex_bc)
            st[f"diag{phase}"] = diag

        def emit_h(tt, phase):
            st = state[tt]
            v, pcur, diag = st["v"], st["pcur"], st[f"diag{phase}"]
            h_ps = pbig.tile([128, D], _F32, tag="h")
            for half in range(2):
                hs = slice(512 * half, 512 * half + 512)
                for n in range(NB):
                    rhs = v[:, n, hs] if n < 8 else pcur[:, hs]
                    nc.tensor.matmul(h_ps[:, hs], lhsT=diag[:, n, :],
                                     rhs=rhs, start=(n == 0), stop=(n == 8))
            st[f"h{phase}"] = h_ps

        def emit_hn(tt, phase):
            st = state[tt]
            h_ps = st[f"h{phase}"]
            # cast h to bf16 unscaled; rmsnorm scale folds into the
            # post-GEMM fused op (GEMM is linear in h)
            ssqh = stats.tile([128, 1], _F32, tag=f"sh{phase}")
            jh = work.tile([128, D], _BF, tag="jh")
            nc.scalar.activation(out=jh, in_=h_ps, func=AF.Square,
                                 accum_out=ssqh)
            rih = rsqrt_dve(ssqh[:, :], 1, f"z{phase}", out_scale=32.0)
            hn = work.tile([128, D], _BF, tag=f"hn{phase}")
            nc.scalar.activation(out=hn, in_=h_ps, func=AF.Copy)
            hnT = work.tile([128, NCH, 128], _BF, tag=f"ht{phase}", bufs=3)
            nc.sync.dma_start_transpose(hnT, hn)
            st[f"rih{phase}"] = rih
            st[f"hnT{phase}"] = hnT

        def emit_gemm(tt, phase):
            st = state[tt]
            hnT = st[f"hnT{phase}"]
            w_sb = wa_sb if phase == 0 else wm_sb
            g_ps = pbig.tile([128, D], _F32, tag="g")
            for half in range(2):
                hs = slice(512 * half, 512 * half + 512)
                for c in range(NCH):
                    nc.tensor.matmul(g_ps[:, hs], lhsT=hnT[:, c, :],
                                     rhs=w_sb[:, c, hs],
                                     start=(c == 0), stop=(c == NCH - 1))
            st[f"g{phase}"] = g_ps

        def emit_pout(tt, phase):
            st = state[tt]
            g_ps, rih = st[f"g{phase}"], st[f"rih{phase}"]
            pcur = st["pcur"]
            ssq, dots = st["ssq"], st["dots"]
            if phase == 0:
                # p1 in bf16 directly; it is the phase-2 residual base
                p1 = work.tile([128, D], _BF, tag="p1")
                nc.vector.scalar_tensor_tensor(
                    out=p1, in0=g_ps, scalar=rih[:, :], in1=pcur,
                    op0=OP.mult, op1=OP.add)
                # refresh block-8 stats for phase 2
                ja = work.tile([128, D], _BF, tag="ja0")
                nc.scalar.activation(out=ja, in_=p1, func=AF.Square,
                                     accum_out=ssq[:, 8:9])
                jv = work.tile([128, D], _BF, tag="jv0")
                nc.vector.scalar_tensor_tensor(
                    out=jv, in0=p1, scalar=1.0, in1=qbc[:, 1, :],
                    op0=OP.mult, op1=OP.mult,
                    accum_out=dots[:, 1, 8:9])
                st["pcur"] = p1
            else:
                pout = work.tile([128, D], _F32, tag="po1")
                nc.vector.scalar_tensor_tensor(
                    out=pout, in0=g_ps, scalar=rih[:, :], in1=pcur,
                    op0=OP.mult, op1=OP.add)
                nc.sync.dma_start(out=out[tt], in_=pout)
                del state[tt]

        # software pipeline over tiles: stats(i) | phase0(i-1) | phase1(i-2)
        # with per-engine interleaving of the two in-flight phases
        for i in range(NT + 2):
            if i == 1:
                emit_weight_loads()
            a = i - 1   # tile in phase 0
            b = i - 2   # tile in phase 1
            if 0 <= a < NT:
                emit_softmax_diag(a, 0)
            if 0 <= b < NT:
                emit_softmax_diag(b, 1)
            if 0 <= a < NT:
                emit_h(a, 0)
            if 0 <= b < NT:
                emit_h(b, 1)
            if i < NT:
                emit_load_stats(i)
            if 0 <= a < NT:
                emit_hn(a, 0)
            if 0 <= b < NT:
                emit_hn(b, 1)
            if 0 <= a < NT:
                emit_gemm(a, 0)
            if 0 <= b < NT:
                emit_gemm(b, 1)
            if 0 <= a < NT:
                emit_pout(a, 0)
            if 0 <= b < NT:
                emit_pout(b, 1)

    nc.compile()
    return nc


def _get_nc():
    if "nc" not in _CACHE:
        _CACHE["nc"] = build_nc()
    return _CACHE["nc"]


def _prepare_in_maps(completed_blocks, partial_block, attn_norm_w, attn_w,
                     mlp_norm_w, mlp_w, attn_res_query, attn_res_norm_w,
                     mlp_res_query, mlp_res_norm_w):
    V = np.ascontiguousarray(
        np.asarray(completed_blocks, np.float32)).reshape(N_BLK, TOK, D)
    P = np.ascontiguousarray(
        np.asarray(partial_block, np.float32)).reshape(TOK, D)
    qwa = np.asarray(attn_res_query, np.float32) * np.asarray(attn_res_norm_w, np.float32)
    qwm = np.asarray(mlp_res_query, np.float32) * np.asarray(mlp_res_norm_w, np.float32)
    WaT = (np.asarray(attn_w, np.float32) * np.asarray(attn_norm_w, np.float32)[None, :]).T
    WmT = (np.asarray(mlp_w, np.float32) * np.asarray(mlp_norm_w, np.float32)[None, :]).T

    qb_host = np.ascontiguousarray(
        np.stack([qwa, qwm], axis=0).astype(bf16))            # [2, D]
    wa_host = np.ascontiguousarray(
        WaT.astype(bf16).reshape(NCH, 128, D).transpose(1, 0, 2))
    wm_host = np.ascontiguousarray(
        WmT.astype(bf16).reshape(NCH, 128, D).transpose(1, 0, 2))

    in_maps = []
    for c in range(NCORES):
        sl = slice(c * TPC, (c + 1) * TPC)
        Vc = V[:, sl, :].astype(bf16)                          # [8, 1024, 1024]
        Pc = P[sl].astype(bf16)                                # [1024, 1024]
        vn_host = np.empty((NT, 128, NB, D), dtype=bf16)
        vn_host[:, :, :8, :] = Vc.reshape(N_BLK, NT, 128, D).transpose(1, 2, 0, 3)
        vn_host[:, :, 8, :] = Pc.reshape(NT, 128, D)
        in_maps.append(dict(vn=vn_host, qb=qb_host, wa=wa_host, wm=wm_host))
    return in_maps


def _run(in_maps, **kw):
    nc = _get_nc()
    return run_bass_kernel_spmd(nc, in_maps, core_ids=list(range(NCORES)), **kw)


def kernel(completed_blocks, partial_block, attn_norm_w, attn_w, mlp_norm_w,
           mlp_w, attn_res_query, attn_res_norm_w, mlp_res_query,
           mlp_res_norm_w, layer_in_block=None, **_unused):
    in_maps = _prepare_in_maps(completed_blocks, partial_block, attn_norm_w,
                               attn_w, mlp_norm_w, mlp_w, attn_res_query,
                               attn_res_norm_w, mlp_res_query, mlp_res_norm_w)
    res = _run(in_maps)
    outs = [np.asarray(r["out"], np.float32).reshape(TPC, D) for r in res.results]
    return np.concatenate(outs, axis=0).reshape(B, T, D)


# revision 17
# speedup vs baseline: 1.0727x; 1.0174x over previous
"""Trainium2 Bass kernel for nn_BlockAttnResTransformerBlock.

Two sequential inter-block-attention sub-layers over 9 blocks (8 completed +
partial).  Per token t (8192 tokens, data-parallel over 8 cores):
  logit_n = <q, V_n[t]> * rsqrt(mean(V_n[t]^2) + eps) / sqrt(D)
  w_n     = exp(logit_n)            (softmax normalizer skipped: rmsnorm of h
                                     is scale-invariant, so it cancels)
  h       = sum_n w_n V_n[t]
  out     = partial[t] + rmsnorm(h) @ W_eff.T    (W_eff = W * norm_w, folded)
Phase 2 repeats with the updated partial and the mlp query/weights.

Engine split (per 128-token tile):
  DVE: 18+1 fused dot-products (scalar_tensor_tensor with accum_out),
       reciprocal for rsqrt, small logit muls
  ACT: 9+1 sum-of-squares (Square + accum), exp, diag(w) builds, h-norm
  PE : diag h-build matmuls, the two DxD GEMMs, residual add folded into the
       GEMM PSUM accumulation via an identity matmul
V ships in ONE bf16 layout (natural [t, n, d], partial packed as block 8);
output is written f32 straight from PSUM.
"""

import numpy as np
import ml_dtypes
from contextlib import ExitStack

import concourse.bass as bass
import concourse.bacc as bacc
import concourse.tile as tile
from concourse import mybir
from concourse.bass_utils import run_bass_kernel_spmd
from concourse.masks import make_identity

bf16 = ml_dtypes.bfloat16

N_BLK = 8          # completed blocks
NB = 9             # + the partial block
B, T, D = 4, 2048, 1024
NCORES = 8
TOK = B * T                  # 8192
TPC = TOK // NCORES          # 1024 tokens per core
NT = TPC // 128              # 8 token-tiles per core
NCH = D // 128               # 8 d-chunks
EPS = 1e-6
INV_SCALE = 1.0 / 32.0       # 1/sqrt(D)

_BF = mybir.dt.bfloat16
_F32 = mybir.dt.float32

_CACHE = {}


def build_nc():
    nc = bacc.Bacc("TRN2", target_bir_lowering=False, debug=False)

    vn = nc.dram_tensor("vn", [NT, 128, NB, D], _BF, kind="ExternalInput")
    qb = nc.dram_tensor("qb", [2, D], _BF, kind="ExternalInput")
    wa = nc.dram_tensor("wa", [128, NCH, D], _BF, kind="ExternalInput")
    wm = nc.dram_tensor("wm", [128, NCH, D], _BF, kind="ExternalInput")
    out = nc.dram_tensor("out", [NT, 128, D], _F32, kind="ExternalOutput")

    AF = mybir.ActivationFunctionType
    OP = mybir.AluOpType

    with tile.TileContext(nc) as tc, ExitStack() as ctx:
        consts = ctx.enter_context(tc.tile_pool(name="consts", bufs=1))
        vin = ctx.enter_context(tc.tile_pool(name="vin", bufs=4))
        stats = ctx.enter_context(tc.tile_pool(name="stats", bufs=6))
        work = ctx.enter_context(tc.tile_pool(name="work", bufs=2))
        pbig = ctx.enter_context(tc.tile_pool(name="pbig", bufs=2, space="PSUM"))

        ident = consts.tile([128, 128], _BF)
        make_identity(nc, ident)
        ident9 = consts.tile([128, NB, 128], _BF)
        for n in range(NB):
            nc.scalar.activation(out=ident9[:, n, :], in_=ident, func=AF.Copy)
        qbc = consts.tile([128, 2, D], _BF)
        qb_ap = qb[:, :]
        nc.sync.dma_start(out=qbc, in_=bass.AP(
            tensor=qb_ap.tensor, offset=qb_ap.offset,
            ap=[[0, 128]] + list(qb_ap.ap)))
        wa_sb = consts.tile([128, NCH, D], _BF)
        wm_sb = consts.tile([128, NCH, D], _BF)

        def emit_weight_loads():
            nc.sync.dma_start(out=wa_sb, in_=wa[:, :, :])
            nc.sync.dma_start(out=wm_sb, in_=wm[:, :, :])

        _I32 = mybir.dt.int32

        def rsqrt_dve(src_ap, w, tag, out_scale=1.0):
            """y ~= out_scale * rsqrt(src) on DVE (Quake seed + 1 Newton).

            src must be safely > 0 (sum of many squares here). out_scale is
            folded into the Newton constants for free."""
            i32 = stats.tile([128, w], _I32, tag=tag + "i")
            nc.vector.tensor_scalar(out=i32, in0=src_ap.bitcast(_I32),
                                    scalar1=1, scalar2=-1,
                                    op0=OP.logical_shift_right,
                                    op1=OP.bitwise_xor)
            y0i = stats.tile([128, w], _I32, tag=tag + "y0")
            nc.vector.tensor_scalar(out=y0i, in0=i32, scalar1=1597463008,
                                    scalar2=None, op0=OP.add)
            ycur = y0i.bitcast(_F32)
            t1 = stats.tile([128, w], _F32, tag=tag + "t")
            ynext = stats.tile([128, w], _F32, tag=tag + "yn")
            nc.vector.tensor_mul(out=t1, in0=ycur, in1=ycur)
            nc.vector.tensor_mul(out=t1, in0=t1, in1=src_ap)
            nc.vector.tensor_scalar(out=t1, in0=t1,
                                    scalar1=-0.5 * out_scale,
                                    scalar2=1.5 * out_scale,
                                    op0=OP.mult, op1=OP.add)
            nc.vector.tensor_mul(out=ynext, in0=ycur, in1=t1)
            return ynext

        state = {}

        def emit_load_stats(tt):
            v = vin.tile([128, NB, D], _BF, tag="v")
            for part in range(3):
                ns = slice(3 * part, 3 * part + 3)
                nc.sync.dma_start(out=v[:, ns, :], in_=vn[tt][:, ns, :])
            ssq = stats.tile([128, NB], _F32, tag="ssq")
            dots = stats.tile([128, 2, NB], _F32, tag="dots")

            # per-block reductions: ssq on ACT, dots split DVE/ACT
            for n in range(NB):
                ja = work.tile([128, D], _BF, tag=f"ja{n % 2}")
                nc.scalar.activation(out=ja, in_=v[:, n, :], func=AF.Square,
                                     accum_out=ssq[:, n:n + 1])
            for qi in range(2):
                for n in range(NB):
                    if (qi * NB + n) % 4 < 3:  # 14 on DVE-stt, 4 via ACT
                        jv = work.tile([128, D], _BF, tag=f"jv{n % 2}")
                        nc.vector.scalar_tensor_tensor(
                            out=jv, in0=v[:, n, :], scalar=1.0,
                            in1=qbc[:, qi, :], op0=OP.mult, op1=OP.mult,
                            accum_out=dots[:, qi, n:n + 1])
                    else:
                        pr = work.tile([128, D], _BF, tag=f"pv{n % 2}", bufs=2)
                        nc.vector.tensor_mul(out=pr, in0=v[:, n, :],
                                             in1=qbc[:, qi, :])
                        jb = work.tile([128, D], _BF, tag=f"ja{n % 2}")
                        nc.scalar.activation(out=jb, in_=pr, func=AF.Copy,
                                             accum_out=dots[:, qi, n:n + 1])
            state[tt] = dict(v=v, ssq=ssq, dots=dots, pcur=v[:, 8, :])

        def emit_softmax_diag(tt, phase):
            st = state[tt]
            ssq, dots = st["ssq"], st["dots"]
            # w_n = exp(dot_n * rsqrt(ssq_n/D) / 32) = exp(dot_n*rsqrt(ssq))
            rinv = rsqrt_dve(ssq[:, :], NB, f"r{phase}")
            lg = stats.tile([128, NB], _F32, tag=f"lg{phase}")
            nc.vector.tensor_mul(out=lg, in0=dots[:, phase, :], in1=rinv)
            ex = stats.tile([128, NB], _F32, tag=f"ex{phase}")
            nc.scalar.activation(out=ex, in_=lg, func=AF.Exp)
            # diag(w_n): ACT copy-scales (phase 0) / DVE broadcast (phase 1)
            diag = work.tile([128, NB, 128], _BF, tag=f"dg{phase}", bufs=3)
            if phase == 0:
                for n in range(NB):
                    nc.scalar.activation(out=diag[:, n, :], in_=ident,
                                         func=AF.Copy, scale=ex[:, n:n + 1])
            else:
                ex_ap = ex[:, :]
                ex_bc = bass.AP(tensor=ex_ap.tensor, offset=ex_ap.offset,
                                ap=list(ex_ap.ap) + [[0, 128]])
                nc.vector.tensor_mul(out=diag, in0=ident9, in1=ex_bc)
            st[f"diag{phase}"] = diag

        def emit_h(tt, phase):
            st = state[tt]
            v, pcur, diag = st["v"], st["pcur"], st[f"diag{phase}"]
            h_ps = pbig.tile([128, D], _F32, tag="h")
            for half in range(2):
                hs = slice(512 * half, 512 * half + 512)
                for n in range(NB):
                    rhs = v[:, n, hs] if n < 8 else pcur[:, hs]
                    nc.tensor.matmul(h_ps[:, hs], lhsT=diag[:, n, :],
                                     rhs=rhs, start=(n == 0), stop=(n == 8))
            st[f"h{phase}"] = h_ps

        def emit_hn(tt, phase):
            st = state[tt]
            h_ps = st[f"h{phase}"]
            # cast h to bf16 unscaled; rmsnorm scale folds into the
            # post-GEMM fused op (GEMM is linear in h)
            ssqh = stats.tile([128, 1], _F32, tag=f"sh{phase}")
            jh = work.tile([128, D], _BF, tag="jh")
            nc.scalar.activation(out=jh, in_=h_ps, func=AF.Square,
                                 accum_out=ssqh)
            rih = rsqrt_dve(ssqh[:, :], 1, f"z{phase}", out_scale=32.0)
            hn = work.tile([128, D], _BF, tag=f"hn{phase}", bufs=3)
            nc.scalar.activation(out=hn, in_=h_ps, func=AF.Copy)
            hnT = work.tile([128, NCH, 128], _BF, tag=f"ht{phase}", bufs=3)
            nc.sync.dma_start_transpose(hnT, hn)
            st[f"rih{phase}"] = rih
            st[f"hnT{phase}"] = hnT

        def emit_gemm(tt, phase):
            st = state[tt]
            hnT = st[f"hnT{phase}"]
            w_sb = wa_sb if phase == 0 else wm_sb
            g_ps = pbig.tile([128, D], _F32, tag="g")
            for half in range(2):
                hs = slice(512 * half, 512 * half + 512)
                for c in range(NCH):
                    nc.tensor.matmul(g_ps[:, hs], lhsT=hnT[:, c, :],
                                     rhs=w_sb[:, c, hs],
                                     start=(c == 0), stop=(c == NCH - 1))
            st[f"g{phase}"] = g_ps

        def emit_pout(tt, phase):
            st = state[tt]
            g_ps, rih = st[f"g{phase}"], st[f"rih{phase}"]
            pcur = st["pcur"]
            ssq, dots = st["ssq"], st["dots"]
            if phase == 0:
                # p1 in bf16 directly; it is the phase-2 residual base
                p1 = work.tile([128, D], _BF, tag="p1")
                nc.vector.scalar_tensor_tensor(
                    out=p1, in0=g_ps, scalar=rih[:, :], in1=pcur,
                    op0=OP.mult, op1=OP.add)
                # refresh block-8 stats for phase 2
                ja = work.tile([128, D], _BF, tag="ja0")
                nc.scalar.activation(out=ja, in_=p1, func=AF.Square,
                                     accum_out=ssq[:, 8:9])
                jv = work.tile([128, D], _BF, tag="jv0")
                nc.vector.scalar_tensor_tensor(
                    out=jv, in0=p1, scalar=1.0, in1=qbc[:, 1, :],
                    op0=OP.mult, op1=OP.mult,
                    accum_out=dots[:, 1, 8:9])
                st["pcur"] = p1
            else:
                pout = work.tile([128, D], _F32, tag="po1")
                nc.vector.scalar_tensor_tensor(
                    out=pout, in0=g_ps, scalar=rih[:, :], in1=pcur,
                    op0=OP.mult, op1=OP.add)
                nc.sync.dma_start(out=out[tt], in_=pout)
                del state[tt]

        # software pipeline over tiles: stats(i) | phase0(i-1) | phase1(i-2)
        # with per-engine interleaving of the two in-flight phases
        for i in range(NT + 2):
            if i == 1:
                emit_weight_loads()
            a = i - 1   # tile in phase 0
            b = i - 2   # tile in phase 1
            if 0 <= a < NT:
                emit_softmax_diag(a, 0)
            if 0 <= b < NT:
                emit_softmax_diag(b, 1)
            if 0 <= a < NT:
                emit_h(a, 0)
            if 0 <= b < NT:
                emit_h(b, 1)
            if i < NT:
                emit_load_stats(i)
            if 0 <= a < NT:
                emit_hn(a, 0)
            if 0 <= b < NT:
                emit_hn(b, 1)
            if 0 <= a < NT:
                emit_gemm(a, 0)
            if 0 <= b < NT:
                emit_gemm(b, 1)
            if 0 <= a < NT:
                emit_pout(a, 0)
            if 0 <= b < NT:
                emit_pout(b, 1)

    nc.compile()
    return nc


def _get_nc():
    if "nc" not in _CACHE:
        _CACHE["nc"] = build_nc()
    return _CACHE["nc"]


def _prepare_in_maps(completed_blocks, partial_block, attn_norm_w, attn_w,
                     mlp_norm_w, mlp_w, attn_res_query, attn_res_norm_w,
                     mlp_res_query, mlp_res_norm_w):
    V = np.ascontiguousarray(
        np.asarray(completed_blocks, np.float32)).reshape(N_BLK, TOK, D)
    P = np.ascontiguousarray(
        np.asarray(partial_block, np.float32)).reshape(TOK, D)
    qwa = np.asarray(attn_res_query, np.float32) * np.asarray(attn_res_norm_w, np.float32)
    qwm = np.asarray(mlp_res_query, np.float32) * np.asarray(mlp_res_norm_w, np.float32)
    WaT = (np.asarray(attn_w, np.float32) * np.asarray(attn_norm_w, np.float32)[None, :]).T
    WmT = (np.asarray(mlp_w, np.float32) * np.asarray(mlp_norm_w, np.float32)[None, :]).T

    qb_host = np.ascontiguousarray(
        np.stack([qwa, qwm], axis=0).astype(bf16))            # [2, D]
    wa_host = np.ascontiguousarray(
        WaT.astype(bf16).reshape(NCH, 128, D).transpose(1, 0, 2))
    wm_host = np.ascontiguousarray(
        WmT.astype(bf16).reshape(NCH, 128, D).transpose(1, 0, 2))

    in_maps = []
    for c in range(NCORES):
        sl = slice(c * TPC, (c + 1) * TPC)
        Vc = V[:, sl, :].astype(bf16)                          # [8, 1024, 1024]
        Pc = P[sl].astype(bf16)                                # [1024, 1024]
        vn_host = np.empty((NT, 128, NB, D), dtype=bf16)
        vn_host[:, :, :8, :] = Vc.reshape(N_BLK, NT, 128, D).transpose(1, 2, 0, 3)
        vn_host[:, :, 8, :] = Pc.reshape(NT, 128, D)
        in_maps.append(dict(vn=vn_host, qb=qb_host, wa=wa_host, wm=wm_host))
    return in_maps


def _run(in_maps, **kw):
    nc = _get_nc()
    return run_bass_kernel_spmd(nc, in_maps, core_ids=list(range(NCORES)), **kw)


def kernel(completed_blocks, partial_block, attn_norm_w, attn_w, mlp_norm_w,
           mlp_w, attn_res_query, attn_res_norm_w, mlp_res_query,
           mlp_res_norm_w, layer_in_block=None, **_unused):
    in_maps = _prepare_in_maps(completed_blocks, partial_block, attn_norm_w,
                               attn_w, mlp_norm_w, mlp_w, attn_res_query,
                               attn_res_norm_w, mlp_res_query, mlp_res_norm_w)
    res = _run(in_maps)
    outs = [np.asarray(r["out"], np.float32).reshape(TPC, D) for r in res.results]
    return np.concatenate(outs, axis=0).reshape(B, T, D)
